# revision 6
# baseline (speedup 1.0000x reference)
"""Trainium2 Bass kernel for a StyleGAN-style modulated conv2d.

Reference math (see problem statement):
    w  = kernel * he_std                       # equalized-lr
    s  = style @ w_mod + b_mod + 1             # [B, cin]
    s  = s / max|s|                            # global max-abs over [B, cin]
    w  = w * s[0][None, None, :, None]         # style[0] only -> one shared weight
    d  = rsqrt(sum(w^2, (0,1,2)) + 1e-8)
    w  = w * d
    y  = conv2d_same(x, w) + noise*(ns/2) + bias
    y  = lrelu(y, 0.2) * sqrt(2)

Because only style[0] modulates, the effective 3x3x128x128 weight is identical
for every batch element, so the device work is a plain 3x3 conv. The tiny
modulation math (a 512x128 matvec + norms, ~1e-6 of total FLOPs) is folded on
the host while sharding; the conv + activation run on 8 NeuronCores,
data-parallel over batch (1 image per core).

Device strategy per core (v2 — edge-tuned; steady state was already at the
2.365 GHz PE roofline with zero inter-matmul gaps):
  - x pre-padded/transposed on host to [cin=128, 258*258] bf16 and held in ONE
    persistent SBUF tile (130 KB/partition). It is DMA'd in 33 ascending row
    pieces that are explicitly serialized: a 1-element GpSimd op between
    consecutive pieces reads a byte of piece i (RAW) and a byte of piece i+1's
    range (WAR), forcing piece i+1's transfer to start only after piece i
    lands. Without this, concurrent piece DMAs round-robin at packet
    granularity and the first piece completes ~12 us late (measured), idling
    the PE. First piece is only 4 rows so matmuls start ~9 us earlier.
  - PE warm-up: 8 dummy matmuls on a memset tile right at kernel start keep
    the HAM governor from holding the PE at 1.2 GHz for the first real groups.
  - 3x3 conv = 9 accumulating matmuls per PSUM group: lhsT = w[cin,cout] per
    tap, rhs = 2 output rows x 256 cols (FD=512, one PSUM bank) per group.
  - Epilogue: y = sqrt2*lrelu(psum + bias, 0.2) via Relu/Identity ACT passes +
    DVE add; output tile is bf16 (halves out-DMA; ~1e-3 rel err, gate 2e-2).
    The final out-tile uses a DVE+ACT parallel variant to shorten the drain.
  - Out DMAs issue on the ACT HWDGE ring so the serialized Sync ring
    (in-pieces) can never stall them.
  - Host transposes [cout, H*W] bf16 back to NHWC fp32.
"""

import math
from contextlib import ExitStack

import ml_dtypes
import numpy as np

import concourse.bacc as bacc
import concourse.bass as bass
import concourse.mybir as mybir
import concourse.tile as tile
from concourse.bass_utils import run_bass_kernel_spmd

B, H, W, CIN, COUT, KK, SDIM = 8, 256, 256, 128, 128, 3, 512
HP, WP = H + 2, W + 2  # zero-padded spatial dims (SAME padding for 3x3)
N_CORES = 8
OUT_TILE_ROWS = 8           # rows per output tile (8*256*2B = 4KB/part bf16)
N_OUT_TILES = H // OUT_TILE_ROWS  # 32
GROUP_ROWS = 2              # output rows per PSUM group (2*256 = 512 = 1 bank)

BF16 = mybir.dt.bfloat16
F32 = mybir.dt.float32
SQRT2 = float(np.sqrt(np.float32(2.0)))

# x DMA piece row boundaries: tiny first piece for an early first matmul,
# then 8-row pieces. Consecutive pieces are chained (see module docstring).
_PIECE_BOUNDS = [0, 4] + list(range(12, HP, 8)) + [HP]


def _effective_weight(style, kernel, w_mod, b_mod):
    """Exactly the reference weight math, in fp32 numpy."""
    style = np.asarray(style, np.float32)
    kernel = np.asarray(kernel, np.float32)
    w_mod = np.asarray(w_mod, np.float32)
    b_mod = np.asarray(b_mod, np.float32)

    he_std = np.float32(1.0) / np.sqrt(np.float32(KK * KK * CIN))
    w = kernel * he_std
    s = (style @ w_mod + b_mod + np.float32(1.0)).astype(np.float32)
    s = s * (np.float32(1.0) / np.max(np.abs(s)))
    w = w * s[0][None, None, :, None]
    d = np.float32(1.0) / np.sqrt(
        np.sum(np.square(w), axis=(0, 1, 2), dtype=np.float32) + np.float32(1e-8)
    )
    w = w * d[None, None, None, :]
    return w.astype(np.float32)  # [3, 3, cin, cout]


def _build_program(with_noise: bool, fast_epi: bool):
    # Bacc (not raw Bass): its compile() splits multi-sem sync waits into
    # event semaphores — TRN2 allows at most one wait per instruction.
    nc = bacc.Bacc(trn_type="TRN2")
    x = nc.declare_dram_parameter("x", [CIN, HP * WP], BF16, isOutput=False)
    w = nc.declare_dram_parameter("w", [CIN, 9 * COUT], BF16, isOutput=False)
    # ab[:,0] = bias*0.8*sqrt2, ab[:,1] = bias*0.2*sqrt2 (lrelu decomposition)
    ab = nc.declare_dram_parameter("ab", [COUT, 2], F32, isOutput=False)
    if with_noise:
        nz = nc.declare_dram_parameter("nz", [1, H * W], BF16, isOutput=False)
        ones = nc.declare_dram_parameter("ones", [1, COUT], BF16, isOutput=False)
    y = nc.declare_dram_parameter("y", [COUT, H * W], BF16, isOutput=True)

    with ExitStack() as ctx:
        tc = ctx.enter_context(tile.TileContext(nc))
        consts = ctx.enter_context(tc.tile_pool(name="consts", bufs=1))
        opool = ctx.enter_context(tc.tile_pool(name="out", bufs=3))
        pspool = ctx.enter_context(tc.tile_pool(name="ps", bufs=6, space="PSUM"))
        wupool = ctx.enter_context(tc.tile_pool(name="wu", bufs=1, space="PSUM"))
        tpool = ctx.enter_context(tc.tile_pool(name="tmp", bufs=6))

        # --- x: one persistent tile, DMA'd in ascending row pieces on the
        # GpSimd SWDGE ring (GpSimd is otherwise idle, so the gating waits
        # below can stall it freely — unlike Sync/ACT, whose HWDGE triggers
        # sit in streams that also carry out-DMAs / activations). Issued
        # before anything else: the head piece's arrival gates the first
        # real matmul.
        xt = consts.tile([CIN, HP * WP], BF16)
        scr = consts.tile([1, 64], F32)
        pieces = list(zip(_PIECE_BOUNDS[:-1], _PIECE_BOUNDS[1:]))
        for a, b in pieces[:2]:  # head pieces: ungated, needed immediately
            nc.gpsimd.dma_start(xt[:, a * WP : b * WP], x[:, a * WP : b * WP])

        # --- PE warm-up: HAM holds the PE at 1.2 GHz until it has seen a few
        # us of sustained matmul activity; an idle gap before the first real
        # group forfeits the credit, and all-zero operands barely register on
        # the (power-based) activity monitor — so fill with varying garbage
        # via iota and bridge until the first x piece lands (~10.5 us).
        wz = consts.tile([CIN, 512], BF16)
        nc.gpsimd.iota(wz[:], [[1, 512]], base=1, channel_multiplier=7,
                       allow_small_or_imprecise_dtypes=True)
        wups = wupool.tile([128, 256], F32)
        for _ in range(14):
            nc.tensor.matmul(wups[:], wz[:, 0:128], wz[:, 256:512],
                             start=True, stop=True)

        wt = consts.tile([CIN, 9 * COUT], BF16)
        # tap 0 first: the very first conv matmul needs only wt[:, 0:COUT],
        # so don't make it wait on the full 295KB weight transfer.
        nc.sync.dma_start(wt[:, 0:COUT], w[:, 0:COUT])
        nc.sync.dma_start(wt[:, COUT:], w[:, COUT:])
        abt = consts.tile([COUT, 2], F32)
        nc.sync.dma_start(abt[:], ab[:])
        if with_noise:
            onest = consts.tile([1, COUT], BF16)
            nc.sync.dma_start(onest[:], ones[:])
            nzt = consts.tile([1, H * W], BF16)
            nc.sync.dma_start(nzt[:], nz[:])

        xv = xt[:].rearrange("p (r c) -> p r c", c=WP)
        for th in range(N_OUT_TILES):
            ot = opool.tile([COUT, OUT_TILE_ROWS * W], BF16)
            for g in range(OUT_TILE_ROWS // GROUP_ROWS):
                rr = th * OUT_TILE_ROWS + g * GROUP_ROWS  # output row
                ps = pspool.tile([COUT, GROUP_ROWS * W], F32)
                for t in range(9):
                    dh, dw = divmod(t, 3)
                    rhs = xv[:, rr + dh : rr + dh + GROUP_ROWS, dw : dw + W]
                    nc.tensor.matmul(
                        ps[:],
                        wt[:, t * COUT : (t + 1) * COUT],
                        rhs,
                        start=(t == 0),
                        stop=(t == 8 and not with_noise),
                    )
                if with_noise:
                    nc.tensor.matmul(
                        ps[:],
                        onest[:],
                        nzt[:, rr * W : (rr + GROUP_ROWS) * W],
                        start=False,
                        stop=True,
                    )
                # sqrt2*lrelu(z,0.2) = Relu(0.8*sqrt2*z) + 0.2*sqrt2*z,
                # z = psum + bias. ACT's Lrelu LUT has a fixed 0.01
                # slope (alpha is ignored), so build it from exact ops.
                oslice = ot[:, g * GROUP_ROWS * W : (g + 1) * GROUP_ROWS * W]
                t1 = tpool.tile([COUT, GROUP_ROWS * W], F32)
                if fast_epi and th == N_OUT_TILES - 1:
                    # Final tile: run the relu branch on DVE in parallel with
                    # the ACT pass to shorten the kernel-tail drain. Valid
                    # only for bias == 0 (relu before bias-add otherwise).
                    nc.vector.tensor_scalar(
                        t1[:], ps[:], 0.0, 0.8 * SQRT2,
                        op0=mybir.AluOpType.max, op1=mybir.AluOpType.mult,
                    )
                else:
                    nc.scalar.activation(
                        t1[:],
                        ps[:],
                        mybir.ActivationFunctionType.Relu,
                        bias=abt[:, 0:1],
                        scale=0.8 * SQRT2,
                    )
                nc.scalar.activation(
                    oslice,
                    ps[:],
                    mybir.ActivationFunctionType.Identity,
                    bias=abt[:, 1:2],
                    scale=0.2 * SQRT2,
                )
                nc.vector.tensor_add(oslice, oslice, t1[:])
                if th == N_OUT_TILES - 1:
                    # Final tile: per-group out DMAs so the last transfer
                    # (and its ~2.5 us completion latency) covers 2 rows,
                    # not 8.
                    nc.sync.dma_start(
                        y[:, rr * W : (rr + GROUP_ROWS) * W], oslice
                    )
            if th < N_OUT_TILES - 1:
                row = th * OUT_TILE_ROWS
                nc.sync.dma_start(
                    y[:, row * W : (row + OUT_TILE_ROWS) * W], ot[:]
                )
            # Pace the x stream off compute: gate piece th+2's DMA on this
            # tile's first output rows (1-elem GpSimd op: RAW on ot, WAR
            # against the piece's DMA write). Pieces then land ~5 us before
            # their first reader while never crowding the DMA ring — an
            # unpaced prefetch measurably delays the head piece and idles
            # the PE for ~6 us at kernel start.
            k = th + 2
            if k < len(pieces):
                a, b = pieces[k]
                nc.gpsimd.tensor_add(
                    scr[:, k : k + 1],
                    ot[0:1, 0:1],
                    xt[0:1, a * WP : a * WP + 1],
                )
                nc.gpsimd.dma_start(
                    xt[:, a * WP : b * WP], x[:, a * WP : b * WP]
                )
    nc.finalize()  # Bacc.compile(): reg alloc + split multi-sem waits (TRN2)
    return nc


def _run(inputs, trace=False, **spmd_kwargs):
    x = np.asarray(inputs["x"])
    noise_strength = float(np.asarray(inputs["noise_strength"]).reshape(-1)[0])
    bias = np.asarray(inputs["bias"], np.float32)

    w_eff = _effective_weight(
        inputs["style"], inputs["kernel"], inputs["w_mod"], inputs["b_mod"]
    )
    # [3,3,cin,cout] -> [cin, tap*cout], tap-major free dim
    w_dev = np.ascontiguousarray(
        w_eff.transpose(2, 0, 1, 3).reshape(CIN, 9 * COUT)
    ).astype(ml_dtypes.bfloat16)

    # Pad + NHWC->NCHW per image, cast bf16. Zero borders bake in SAME padding.
    x_pad = np.zeros((B, CIN, HP, WP), dtype=ml_dtypes.bfloat16)
    x_pad[:, :, 1 : H + 1, 1 : W + 1] = x.transpose(0, 3, 1, 2).astype(
        ml_dtypes.bfloat16
    )

    ab = np.stack(
        [
            bias * np.float32(0.8 * SQRT2),
            bias * np.float32(0.2 * SQRT2),
        ],
        axis=1,
    ).astype(np.float32)  # [COUT, 2]

    with_noise = noise_strength != 0.0
    fast_epi = not np.any(bias)
    in_maps = []
    for b in range(B):
        m = {
            "x": np.ascontiguousarray(x_pad[b].reshape(CIN, HP * WP)),
            "w": w_dev,
            "ab": ab,
        }
        if with_noise:
            nzb = np.asarray(inputs["noise"], np.float32)[b, :, :, 0] * np.float32(
                noise_strength / 2.0
            )
            m["nz"] = nzb.reshape(1, H * W).astype(ml_dtypes.bfloat16)
            m["ones"] = np.ones((1, COUT), dtype=ml_dtypes.bfloat16)
        in_maps.append(m)

    nc = _build_program(with_noise, fast_epi)
    res = run_bass_kernel_spmd(
        nc, in_maps, list(range(N_CORES)), trace=trace, **spmd_kwargs
    )

    out = np.empty((B, H, W, COUT), dtype=np.float32)
    for b in range(B):
        yb = np.asarray(res.results[b]["y"]).astype(np.float32)
        out[b] = yb.reshape(COUT, H, W).transpose(1, 2, 0)
    return out, res


def kernel(**inputs):
    out, _ = _run(inputs)
    return out


# revision 7
# speedup vs baseline: 1.0199x; 1.0199x over previous
"""Trainium2 Bass kernel for a StyleGAN-style modulated conv2d.

Reference math (see problem statement):
    w  = kernel * he_std                       # equalized-lr
    s  = style @ w_mod + b_mod + 1             # [B, cin]
    s  = s / max|s|                            # global max-abs over [B, cin]
    w  = w * s[0][None, None, :, None]         # style[0] only -> one shared weight
    d  = rsqrt(sum(w^2, (0,1,2)) + 1e-8)
    w  = w * d
    y  = conv2d_same(x, w) + noise*(ns/2) + bias
    y  = lrelu(y, 0.2) * sqrt(2)

Because only style[0] modulates, the effective 3x3x128x128 weight is identical
for every batch element, so the device work is a plain 3x3 conv. The tiny
modulation math (a 512x128 matvec + norms, ~1e-6 of total FLOPs) is folded on
the host while sharding; the conv + activation run on 8 NeuronCores,
data-parallel over batch (1 image per core).

Device strategy per core (v2 — edge-tuned; steady state was already at the
2.365 GHz PE roofline with zero inter-matmul gaps):
  - x pre-padded/transposed on host to [cin=128, 258*258] bf16 and held in ONE
    persistent SBUF tile (130 KB/partition). It is DMA'd in 33 ascending row
    pieces that are explicitly serialized: a 1-element GpSimd op between
    consecutive pieces reads a byte of piece i (RAW) and a byte of piece i+1's
    range (WAR), forcing piece i+1's transfer to start only after piece i
    lands. Without this, concurrent piece DMAs round-robin at packet
    granularity and the first piece completes ~12 us late (measured), idling
    the PE. First piece is only 4 rows so matmuls start ~9 us earlier.
  - PE warm-up: 8 dummy matmuls on a memset tile right at kernel start keep
    the HAM governor from holding the PE at 1.2 GHz for the first real groups.
  - 3x3 conv = 9 accumulating matmuls per PSUM group: lhsT = w[cin,cout] per
    tap, rhs = 2 output rows x 256 cols (FD=512, one PSUM bank) per group.
  - Epilogue: y = sqrt2*lrelu(psum + bias, 0.2) via Relu/Identity ACT passes +
    DVE add; output tile is bf16 (halves out-DMA; ~1e-3 rel err, gate 2e-2).
    The final out-tile uses a DVE+ACT parallel variant to shorten the drain.
  - Out DMAs issue on the ACT HWDGE ring so the serialized Sync ring
    (in-pieces) can never stall them.
  - Host transposes [cout, H*W] bf16 back to NHWC fp32.
"""

import math
from contextlib import ExitStack

import ml_dtypes
import numpy as np

import concourse.bacc as bacc
import concourse.bass as bass
import concourse.mybir as mybir
import concourse.tile as tile
from concourse.bass_utils import run_bass_kernel_spmd

B, H, W, CIN, COUT, KK, SDIM = 8, 256, 256, 128, 128, 3, 512
HP, WP = H + 2, W + 2  # zero-padded spatial dims (SAME padding for 3x3)
N_CORES = 8
OUT_TILE_ROWS = 8           # rows per output tile (8*256*2B = 4KB/part bf16)
N_OUT_TILES = H // OUT_TILE_ROWS  # 32
GROUP_ROWS = 2              # output rows per PSUM group (2*256 = 512 = 1 bank)

BF16 = mybir.dt.bfloat16
F32 = mybir.dt.float32
SQRT2 = float(np.sqrt(np.float32(2.0)))

# x DMA piece row boundaries: tiny first piece for an early first matmul,
# then 8-row pieces. Consecutive pieces are chained (see module docstring).
_PIECE_BOUNDS = [0, 4] + list(range(12, HP, 8)) + [HP]


def _effective_weight(style, kernel, w_mod, b_mod):
    """Exactly the reference weight math, in fp32 numpy."""
    style = np.asarray(style, np.float32)
    kernel = np.asarray(kernel, np.float32)
    w_mod = np.asarray(w_mod, np.float32)
    b_mod = np.asarray(b_mod, np.float32)

    he_std = np.float32(1.0) / np.sqrt(np.float32(KK * KK * CIN))
    w = kernel * he_std
    s = (style @ w_mod + b_mod + np.float32(1.0)).astype(np.float32)
    s = s * (np.float32(1.0) / np.max(np.abs(s)))
    w = w * s[0][None, None, :, None]
    d = np.float32(1.0) / np.sqrt(
        np.sum(np.square(w), axis=(0, 1, 2), dtype=np.float32) + np.float32(1e-8)
    )
    w = w * d[None, None, None, :]
    return w.astype(np.float32)  # [3, 3, cin, cout]


def _build_program(with_noise: bool, fast_epi: bool):
    # Bacc (not raw Bass): its compile() splits multi-sem sync waits into
    # event semaphores — TRN2 allows at most one wait per instruction.
    nc = bacc.Bacc(trn_type="TRN2")
    x = nc.declare_dram_parameter("x", [CIN, HP * WP], BF16, isOutput=False)
    w = nc.declare_dram_parameter("w", [CIN, 9 * COUT], BF16, isOutput=False)
    # ab[:,0] = bias*0.8*sqrt2, ab[:,1] = bias*0.2*sqrt2 (lrelu decomposition)
    ab = nc.declare_dram_parameter("ab", [COUT, 2], F32, isOutput=False)
    if with_noise:
        nz = nc.declare_dram_parameter("nz", [1, H * W], BF16, isOutput=False)
        ones = nc.declare_dram_parameter("ones", [1, COUT], BF16, isOutput=False)
    y = nc.declare_dram_parameter("y", [COUT, H * W], BF16, isOutput=True)

    with ExitStack() as ctx:
        tc = ctx.enter_context(tile.TileContext(nc))
        consts = ctx.enter_context(tc.tile_pool(name="consts", bufs=1))
        opool = ctx.enter_context(tc.tile_pool(name="out", bufs=3))
        pspool = ctx.enter_context(tc.tile_pool(name="ps", bufs=6, space="PSUM"))
        wupool = ctx.enter_context(tc.tile_pool(name="wu", bufs=1, space="PSUM"))
        tpool = ctx.enter_context(tc.tile_pool(name="tmp", bufs=6))

        # --- x: one persistent tile, DMA'd in ascending row pieces on the
        # GpSimd SWDGE ring (GpSimd is otherwise idle, so the gating waits
        # below can stall it freely — unlike Sync/ACT, whose HWDGE triggers
        # sit in streams that also carry out-DMAs / activations). Issued
        # before anything else: the head piece's arrival gates the first
        # real matmul.
        xt = consts.tile([CIN, HP * WP], BF16)
        scr = consts.tile([1, 64], F32)
        pieces = list(zip(_PIECE_BOUNDS[:-1], _PIECE_BOUNDS[1:]))
        for a, b in pieces[:2]:  # head pieces: ungated, needed immediately
            nc.gpsimd.dma_start(xt[:, a * WP : b * WP], x[:, a * WP : b * WP])

        # --- PE warm-up: HAM holds the PE at 1.2 GHz until it has seen a few
        # us of sustained matmul activity; an idle gap before the first real
        # group forfeits the credit, and all-zero operands barely register on
        # the (power-based) activity monitor — so fill with varying garbage
        # via iota and bridge until the first x piece lands (~10.5 us).
        wz = consts.tile([CIN, 512], BF16)
        nc.vector.random(wz[:])  # random bits = max toggle power for HAM
        wups = wupool.tile([128, 256], F32)
        for _ in range(16):
            nc.tensor.matmul(wups[:], wz[:, 0:128], wz[:, 256:512],
                             start=True, stop=True)

        wt = consts.tile([CIN, 9 * COUT], BF16)
        # tap 0 first: the very first conv matmul needs only wt[:, 0:COUT],
        # so don't make it wait on the full 295KB weight transfer.
        nc.sync.dma_start(wt[:, 0:COUT], w[:, 0:COUT])
        nc.sync.dma_start(wt[:, COUT:], w[:, COUT:])
        abt = consts.tile([COUT, 2], F32)
        nc.sync.dma_start(abt[:], ab[:])
        if with_noise:
            onest = consts.tile([1, COUT], BF16)
            nc.sync.dma_start(onest[:], ones[:])
            nzt = consts.tile([1, H * W], BF16)
            nc.sync.dma_start(nzt[:], nz[:])

        xv = xt[:].rearrange("p (r c) -> p r c", c=WP)
        for th in range(N_OUT_TILES):
            ot = opool.tile([COUT, OUT_TILE_ROWS * W], BF16)
            for g in range(OUT_TILE_ROWS // GROUP_ROWS):
                rr = th * OUT_TILE_ROWS + g * GROUP_ROWS  # output row
                ps = pspool.tile([COUT, GROUP_ROWS * W], F32)
                for t in range(9):
                    dh, dw = divmod(t, 3)
                    rhs = xv[:, rr + dh : rr + dh + GROUP_ROWS, dw : dw + W]
                    nc.tensor.matmul(
                        ps[:],
                        wt[:, t * COUT : (t + 1) * COUT],
                        rhs,
                        start=(t == 0),
                        stop=(t == 8 and not with_noise),
                    )
                if with_noise:
                    nc.tensor.matmul(
                        ps[:],
                        onest[:],
                        nzt[:, rr * W : (rr + GROUP_ROWS) * W],
                        start=False,
                        stop=True,
                    )
                # sqrt2*lrelu(z,0.2) = Relu(0.8*sqrt2*z) + 0.2*sqrt2*z,
                # z = psum + bias. ACT's Lrelu LUT has a fixed 0.01
                # slope (alpha is ignored), so build it from exact ops.
                oslice = ot[:, g * GROUP_ROWS * W : (g + 1) * GROUP_ROWS * W]
                t1 = tpool.tile([COUT, GROUP_ROWS * W], F32)
                if fast_epi and th == N_OUT_TILES - 1:
                    # Final tile: run the relu branch on DVE in parallel with
                    # the ACT pass to shorten the kernel-tail drain. Valid
                    # only for bias == 0 (relu before bias-add otherwise).
                    nc.vector.tensor_scalar(
                        t1[:], ps[:], 0.0, 0.8 * SQRT2,
                        op0=mybir.AluOpType.max, op1=mybir.AluOpType.mult,
                    )
                else:
                    nc.scalar.activation(
                        t1[:],
                        ps[:],
                        mybir.ActivationFunctionType.Relu,
                        bias=abt[:, 0:1],
                        scale=0.8 * SQRT2,
                    )
                nc.scalar.activation(
                    oslice,
                    ps[:],
                    mybir.ActivationFunctionType.Identity,
                    bias=abt[:, 1:2],
                    scale=0.2 * SQRT2,
                )
                nc.vector.tensor_add(oslice, oslice, t1[:])
                if th == N_OUT_TILES - 1:
                    # Final tile: per-group out DMAs so the last transfer
                    # (and its ~2.5 us completion latency) covers 2 rows,
                    # not 8.
                    nc.sync.dma_start(
                        y[:, rr * W : (rr + GROUP_ROWS) * W], oslice
                    )
            if th < N_OUT_TILES - 1:
                row = th * OUT_TILE_ROWS
                nc.sync.dma_start(
                    y[:, row * W : (row + OUT_TILE_ROWS) * W], ot[:]
                )
            # Pace the x stream off compute: gate piece th+2's DMA on this
            # tile's first output rows (1-elem GpSimd op: RAW on ot, WAR
            # against the piece's DMA write). Pieces then land ~5 us before
            # their first reader while never crowding the DMA ring — an
            # unpaced prefetch measurably delays the head piece and idles
            # the PE for ~6 us at kernel start.
            k = th + 2
            if k < len(pieces):
                a, b = pieces[k]
                nc.gpsimd.tensor_add(
                    scr[:, k : k + 1],
                    ot[0:1, 0:1],
                    xt[0:1, a * WP : a * WP + 1],
                )
                nc.gpsimd.dma_start(
                    xt[:, a * WP : b * WP], x[:, a * WP : b * WP]
                )
    nc.finalize()  # Bacc.compile(): reg alloc + split multi-sem waits (TRN2)
    return nc


def _run(inputs, trace=False, **spmd_kwargs):
    x = np.asarray(inputs["x"])
    noise_strength = float(np.asarray(inputs["noise_strength"]).reshape(-1)[0])
    bias = np.asarray(inputs["bias"], np.float32)

    w_eff = _effective_weight(
        inputs["style"], inputs["kernel"], inputs["w_mod"], inputs["b_mod"]
    )
    # [3,3,cin,cout] -> [cin, tap*cout], tap-major free dim
    w_dev = np.ascontiguousarray(
        w_eff.transpose(2, 0, 1, 3).reshape(CIN, 9 * COUT)
    ).astype(ml_dtypes.bfloat16)

    # Pad + NHWC->NCHW per image, cast bf16. Zero borders bake in SAME padding.
    x_pad = np.zeros((B, CIN, HP, WP), dtype=ml_dtypes.bfloat16)
    x_pad[:, :, 1 : H + 1, 1 : W + 1] = x.transpose(0, 3, 1, 2).astype(
        ml_dtypes.bfloat16
    )

    ab = np.stack(
        [
            bias * np.float32(0.8 * SQRT2),
            bias * np.float32(0.2 * SQRT2),
        ],
        axis=1,
    ).astype(np.float32)  # [COUT, 2]

    with_noise = noise_strength != 0.0
    fast_epi = not np.any(bias)
    in_maps = []
    for b in range(B):
        m = {
            "x": np.ascontiguousarray(x_pad[b].reshape(CIN, HP * WP)),
            "w": w_dev,
            "ab": ab,
        }
        if with_noise:
            nzb = np.asarray(inputs["noise"], np.float32)[b, :, :, 0] * np.float32(
                noise_strength / 2.0
            )
            m["nz"] = nzb.reshape(1, H * W).astype(ml_dtypes.bfloat16)
            m["ones"] = np.ones((1, COUT), dtype=ml_dtypes.bfloat16)
        in_maps.append(m)

    nc = _build_program(with_noise, fast_epi)
    res = run_bass_kernel_spmd(
        nc, in_maps, list(range(N_CORES)), trace=trace, **spmd_kwargs
    )

    out = np.empty((B, H, W, COUT), dtype=np.float32)
    for b in range(B):
        yb = np.asarray(res.results[b]["y"]).astype(np.float32)
        out[b] = yb.reshape(COUT, H, W).transpose(1, 2, 0)
    return out, res


def kernel(**inputs):
    out, _ = _run(inputs)
    return out
